# revision 31
# baseline (speedup 1.0000x reference)
"""Fused attention-encoding kernel for Trainium2, 8-core batch-parallel SPMD.

Problem (per batch b of 16, p=1024 tokens, d=512 features):
    A[i,j] = wa.P_i + wb.P_j + (wc*P_i).P_j        (si = wa.P_i cancels in softmax)
    SA     = softmax_j(A)
    attn   = SA @ P
    Pc     = [P, attn]
    out    = sigmoid(Pc@w2) * P + sigmoid(Pc@w3) * tanh(Pc@w1)

Strategy: batch-parallel over 8 cores (2 batches/core). Large matmuls run in
fp8 (e4m3, max 240) with DoubleRow perf mode — two K=128 subtiles packed along
the free dim stream 2 rows/cycle, 2x the bf16 rate. Scores and attention
tolerate fp8 directly (softmax smoothing); the gate matmuls are
precision-critical in their P-half, so the z and r gates use bf16 there while
f and all attn-halves stay fp8. Everything accumulates in one fp32 PSUM group
via matched power-of-2 scales (P x32, w x256, attn^T x64, w_attn x128 — all
products 8192, descaled inside the activation).

Each batch is processed as two independent i-halves (softmax is over j, so
any i-range is self-contained): scores -> exp -> rowsum -> attn -> gates per
half. That gives a 4-unit pipeline per core whose emission interleaves unit
u's scores with unit u-1's gates, so the PE stays busy while the scalar
engine runs the exps, and it halves the warm-up bubble and drain tail.

sj = P@wb is computed host-side and folded into the exp as a per-partition
activation bias. The softmax denominator comes from an all-ones DoubleRow
matmul with M=128 so every PSUM partition holds the row sum (no partition
broadcast); sigmoid(x) is evaluated as (1+tanh(x/2))/2 so Exp and Tanh share
one activation-table set (no ACT_TABLE_LOAD thrash), with the affine fix-up
folded into the DVE combine against a host-shipped P/2. Input DMAs are spread
across the sync (~97GB/s), scalar (~87GB/s) and pool (~48GB/s) queues in
need-order, tiled per (k-pair, i-half) so whole-tile dependencies resolve as
early as possible.
"""

import sys

if "/opt/trn_rl_repo" not in sys.path:
    sys.path.insert(0, "/opt/trn_rl_repo")

from contextlib import ExitStack

import ml_dtypes
import numpy as np

import concourse.bass as bass
import concourse.mybir as mybir
import concourse.tile as tile
from concourse import bacc
from concourse.bass_utils import run_bass_kernel_spmd

B, PL, D = 16, 1024, 512
NCORES = 8
BPC = B // NCORES          # batches per core
NI = PL // 128             # token blocks: 8
NQ = NI // 2               # token block pairs: 4
ND = D // 128              # feature chunks: 4
HW = PL // 2               # i-half width: 512
FP32 = mybir.dt.float32
BF16 = mybir.dt.bfloat16
FP8 = mybir.dt.float8e4
AF = mybir.ActivationFunctionType
DR = mybir.MatmulPerfMode.DoubleRow
ALU = mybir.AluOpType
F8 = ml_dtypes.float8_e4m3   # IEEE-style e4m3, max 240 — matches TRN fp8e4
BF = ml_dtypes.bfloat16

DESCALE = 1.0 / 8192.0
NB16 = 8                   # bf16 weight slots: gates 0,1 x 4 chunks
N8 = 8                     # fp8 weight slots: 3 attn-half pairs x2, g2 P pairs x2

_cache = {}


def _build(with_bias: bool):
    nc = bacc.Bacc(
        "TRN2", target_bir_lowering=False, debug=False, num_devices=1
    )
    # pt8: per (k-pair, i-half) tiles; pt16/ph16: per i-half tiles
    pt8_d = nc.dram_tensor("pt8", [BPC, 2, 2, 128, 2 * HW], FP8, kind="ExternalInput").ap()
    pwt8_d = nc.dram_tensor("pwt8", [BPC, 2, 128, 2 * PL], FP8, kind="ExternalInput").ap()
    pt16_d = nc.dram_tensor("pt16", [BPC, 2, 128, ND * HW], BF16, kind="ExternalInput").ap()
    pn8_d = nc.dram_tensor("pn8", [BPC, 128, 2 * NQ * D], FP8, kind="ExternalInput").ap()
    ph16_d = nc.dram_tensor("ph16", [BPC, 2, 128, 4 * D], BF16, kind="ExternalInput").ap()
    sj_d = nc.dram_tensor("sjc", [BPC, 128, NI], FP32, kind="ExternalInput").ap()
    wb16_d = nc.dram_tensor("wb16", [128, NB16 * D], BF16, kind="ExternalInput").ap()
    w8_d = nc.dram_tensor("w8p", [128, 2 * N8 * D], FP8, kind="ExternalInput").ap()
    one_d = nc.dram_tensor("ones8", [128, 2 * 128], FP8, kind="ExternalInput").ap()
    if with_bias:
        b_d = nc.dram_tensor("b32", [3, D], FP32, kind="ExternalInput").ap()
    out_d = nc.dram_tensor("out", [BPC, PL, D], FP32, kind="ExternalOutput").ap()

    with tile.TileContext(nc) as tc, ExitStack() as ctx:
        pool = lambda name, bufs: ctx.enter_context(
            tc.tile_pool(name=name, bufs=bufs)
        )
        const = pool("const", 1)
        pt8p = pool("pt8", 2)
        pwt8p = pool("pwt8", 2)
        pt16p = pool("pt16", 2)
        pn8p = pool("pn8", 2)
        ph16p = pool("ph16", 2)
        sjp = pool("sj", 2)
        e8p = pool("e8", 2 * NQ)
        at8p = pool("at8", 4)
        rbbp = pool("rbb", 4)
        gactp = pool("gact", 2)
        combp = pool("comb", 2)
        # PSUM: pss 4x[128,512] (banks 0-3), psg 3x[128,512] (banks 4-6)
        pssp = ctx.enter_context(tc.tile_pool(name="pss", bufs=4, space="PSUM"))
        psgp = ctx.enter_context(tc.tile_pool(name="psg", bufs=3, space="PSUM"))

        wb16 = const.tile([128, NB16 * D], BF16, tag="wb16")
        w8p_t = const.tile([128, 2, N8 * D], FP8, tag="w8p")
        ones8 = const.tile([128, 2, 128], FP8, tag="ones8")
        if with_bias:
            bb = [
                const.tile([128, D], FP32, tag=f"bias{g}", name=f"bias{g}")
                for g in range(3)
            ]
            btmp = const.tile([1, 3 * D], FP32, tag="btmp")

        def load_weights():
            # wb16 rides sync (issued there after pn8); w8p + ones on scalar
            nc.sync.dma_start(wb16[:], wb16_d)
            nc.scalar.dma_start(w8p_t.rearrange("p t x -> p (t x)"), w8_d)
            nc.scalar.dma_start(ones8.rearrange("p t m -> p (t m)"), one_d)
            if with_bias:
                nc.scalar.dma_start(btmp[:], b_d.rearrange("g e -> (g e)")[None, :])
                for g in range(3):
                    nc.gpsimd.partition_broadcast(
                        bb[g][:], btmp[0:1, g * D : (g + 1) * D]
                    )

        def w8slot(s):
            return w8p_t[:, :, s * D : (s + 1) * D]

        st = {}  # per-batch/-unit tiles carried across the interleaved emission

        def emit_load(lb):
            # scalar ring: pt8 (kp,h) quarter-tiles then weight packs;
            # sync ring: pwt8 kp-halves, pn8, then per-half pt16/ph16 in
            # first-needed order; pool ring: sj (tiny) and batch-0 outputs
            pt8 = {}
            for h in range(2):
                for kp in range(2):
                    t = pt8p.tile([128, 2, HW], FP8, tag=f"pt8_{kp}{h}",
                                  name=f"pt8_{kp}{h}_{lb}")
                    nc.scalar.dma_start(
                        t.rearrange("p t x -> p (t x)"), pt8_d[lb, kp, h]
                    )
                    pt8[(kp, h)] = t
            pwt8 = []
            for kp in range(2):
                t = pwt8p.tile([128, 2, PL], FP8, tag=f"pwt8{kp}",
                               name=f"pwt8{kp}_{lb}")
                nc.sync.dma_start(t.rearrange("p t x -> p (t x)"), pwt8_d[lb, kp])
                pwt8.append(t)
            sj = sjp.tile([128, NI], FP32, tag="sj")
            nc.gpsimd.dma_start(sj[:], sj_d[lb])
            pn8 = pn8p.tile([128, 2, NQ * D], FP8, tag="pn8")
            nc.sync.dma_start(pn8.rearrange("p t x -> p (t x)"), pn8_d[lb])
            if lb == 0:
                load_weights()
            pt16, ph16 = [], []
            for h in range(2):
                t16 = pt16p.tile([128, ND * HW], BF16, tag=f"pt16{h}",
                                 name=f"pt16{h}_{lb}")
                nc.sync.dma_start(t16[:], pt16_d[lb, h])
                pt16.append(t16)
                p16 = ph16p.tile([128, 4 * D], BF16, tag=f"ph16{h}",
                                 name=f"ph16{h}_{lb}")
                nc.sync.dma_start(p16[:], ph16_d[lb, h])
                ph16.append(p16)
            st[lb] = dict(pt8=pt8, pwt8=pwt8, pt16=pt16, pn8=pn8, sj=sj,
                          ph16=ph16)

        def emit_scores(lb, h):
            s = st[lb]
            e8 = []
            for jb in range(NI):
                if jb % 2 == 0:
                    e8.append(
                        e8p.tile([128, 2, HW], FP8, tag="e8",
                                 name=f"e8_{lb}{h}_{jb}")
                    )
                ps_s = pssp.tile([128, HW], FP32, tag="pss",
                                 name=f"pss{lb}{h}_{jb}")
                for kp in range(2):
                    nc.tensor.matmul(
                        ps_s[:],
                        s["pwt8"][kp][:, :, jb * 128 : (jb + 1) * 128],
                        s["pt8"][(kp, h)][:],
                        start=(kp == 0),
                        stop=(kp == 1),
                        perf_mode=DR,
                    )
                nc.scalar.activation(
                    e8[jb // 2][:, jb % 2, :],
                    ps_s[:],
                    AF.Exp,
                    bias=s["sj"][:, jb : jb + 1],
                    scale=DESCALE,
                )
            st[(lb, h)] = dict(e8=e8)

        def emit_attn(lb, h):
            s, u = st[lb], st[(lb, h)]
            e8 = u["e8"]
            # rowsum over j via all-ones matmul with M=128: every PSUM
            # partition gets the sum, so no partition broadcast is needed
            rs = pssp.tile([128, HW], FP32, tag="pss", name=f"psrs{lb}{h}")
            for q in range(NQ):
                nc.tensor.matmul(
                    rs[:],
                    ones8[:],
                    e8[q][:],
                    start=(q == 0),
                    stop=(q == NQ - 1),
                    perf_mode=DR,
                )
            rbb = rbbp.tile([128, HW], FP32, tag="rbb", name=f"rbb{lb}{h}")
            nc.vector.reciprocal_approx_fast(out=rbb[:], in_=rs[:])
            # attn^T per d-chunk; normalize (x2 folds the 64/32 scale shift)
            at8 = at8p.tile([128, ND * HW], FP8, tag="at8", name=f"at8_{lb}{h}")
            pnv = s["pn8"]
            for dc in range(ND):
                ps_a = pssp.tile([128, HW], FP32, tag="pss",
                                 name=f"psa{lb}{h}_{dc}")
                for q in range(NQ):
                    nc.tensor.matmul(
                        ps_a[:],
                        pnv[:, :, q * D + dc * 128 : q * D + (dc + 1) * 128],
                        e8[q][:],
                        start=(q == 0),
                        stop=(q == NQ - 1),
                        perf_mode=DR,
                    )
                nc.vector.scalar_tensor_tensor(
                    out=at8[:, dc * HW : (dc + 1) * HW],
                    in0=ps_a[:],
                    scalar=2.0,
                    in1=rbb[:],
                    op0=ALU.mult,
                    op1=ALU.mult,
                )
            u["at8"] = at8

        def emit_gates(lb, h):
            s, u = st[lb], st[(lb, h)]
            atv = u["at8"].rearrange("p (c i) -> p c i", i=HW)
            pt16v = s["pt16"][h].rearrange("p (c i) -> p c i", i=HW)
            for ib4 in range(4):
                ib = h * 4 + ib4
                cols = slice(ib4 * 128, (ib4 + 1) * 128)
                ps_g = [
                    psgp.tile([128, 512], FP32, tag="psg",
                              name=f"psg{lb}{h}_{ib4}_{g}")
                    for g in range(3)
                ]
                for g in range(3):
                    # P-half: bf16 for z,r (precision), fp8 DR for f
                    if g < 2:
                        psteps = [
                            ("bf16", pt16v[:, c : c + 1, cols], g * 4 + c)
                            for c in range(ND)
                        ]
                    else:
                        psteps = [
                            ("fp8", s["pt8"][(kp, h)][:, :, cols], 6 + kp)
                            for kp in range(2)
                        ]
                    asteps = [
                        ("fp8", atv[:, 2 * kp : 2 * kp + 2, cols], g * 2 + kp)
                        for kp in range(2)
                    ]
                    steps = psteps + asteps
                    for si, (kind, lhsT, slot) in enumerate(steps):
                        if kind == "bf16":
                            nc.tensor.matmul(
                                ps_g[g],
                                lhsT,
                                wb16[:, slot * D : (slot + 1) * D],
                                start=(si == 0),
                                stop=(si == len(steps) - 1),
                            )
                        else:
                            nc.tensor.matmul(
                                ps_g[g],
                                lhsT,
                                w8slot(slot),
                                start=(si == 0),
                                stop=(si == len(steps) - 1),
                                perf_mode=DR,
                            )
                if with_bias:
                    for g in range(3):
                        nc.vector.tensor_add(ps_g[g][:], ps_g[g][:], bb[g][:])
                z32 = gactp.tile([128, D], FP32, tag="z32")
                r32 = gactp.tile([128, D], FP32, tag="r32")
                f32 = gactp.tile([128, D], FP32, tag="f32")
                nc.scalar.activation(z32[:], ps_g[0][:], AF.Tanh, scale=DESCALE)
                nc.scalar.activation(r32[:], ps_g[1][:], AF.Tanh, scale=DESCALE / 2)
                nc.scalar.activation(f32[:], ps_g[2][:], AF.Tanh, scale=DESCALE / 2)
                # out = (1+r')/2*P + (1+f')/2*z  with r'=tanh(gr/2), f'=tanh(gf/2)
                a32 = combp.tile([128, D], FP32, tag="a32")
                nc.vector.scalar_tensor_tensor(
                    out=a32[:], in0=r32[:], scalar=1.0,
                    in1=s["ph16"][h][:, ib4 * D : (ib4 + 1) * D],
                    op0=ALU.add, op1=ALU.mult,
                )
                b32 = combp.tile([128, D], FP32, tag="b32")
                nc.vector.scalar_tensor_tensor(
                    out=b32[:], in0=f32[:], scalar=1.0, in1=z32[:],
                    op0=ALU.add, op1=ALU.mult,
                )
                o32 = combp.tile([128, D], FP32, tag="o32")
                nc.vector.scalar_tensor_tensor(
                    out=o32[:], in0=b32[:], scalar=0.5, in1=a32[:],
                    op0=ALU.mult, op1=ALU.add,
                )
                # batch 0's outputs ride the otherwise-idle pool ring; the
                # final batch's stay on sync so the last store drains fast
                eng = nc.gpsimd if lb == 0 else nc.sync
                eng.dma_start(out_d[lb, ib * 128 : (ib + 1) * 128, :], o32[:])

        # emission order: scores run two units ahead, gates lag one unit —
        # the PE executes unit u+1's scores and unit u-1's gates while the
        # scalar engine works through unit u's exps
        units = [(lb, h) for lb in range(BPC) for h in range(2)]
        emit_load(0)
        emit_scores(*units[0])
        emit_scores(*units[1])
        emit_attn(*units[0])
        emit_gates(*units[0])
        emit_load(1)
        emit_scores(*units[2])
        emit_attn(*units[1])
        emit_gates(*units[1])
        emit_scores(*units[3])
        emit_attn(*units[2])
        emit_gates(*units[2])
        emit_attn(*units[3])
        emit_gates(*units[3])

    nc.compile()
    return nc


def _get_nc(with_bias: bool):
    if with_bias not in _cache:
        _cache[with_bias] = _build(with_bias)
    return _cache[with_bias]


def _q8(x):
    return np.clip(np.asarray(x, np.float32), -240.0, 240.0).astype(F8)


def _prep_in_maps(P, w_atten, w1, w2, w3, b1, b2, b3):
    P = np.ascontiguousarray(np.asarray(P, dtype=np.float32))
    w_atten = np.asarray(w_atten, dtype=np.float32)
    wb, wc = w_atten[D : 2 * D], w_atten[2 * D :]

    P8 = _q8(P * 32.0)                       # [B, PL, D] fp8

    # pt8[b, kp, h, p, c2, i] = P8[b, h*512+i, (2kp+c2)*128+p]
    pt8 = np.ascontiguousarray(
        P8.reshape(B, 2, HW, 2, 2, 128).transpose(0, 3, 1, 5, 4, 2)
        .reshape(B, 2, 2, 128, 2 * HW)
    )
    # pwt8[b, kp, p, c2, j] = fp8(P*wc*256)[b, j, (2kp+c2)*128+p]
    PW8 = _q8(P * wc * 256.0)
    pwt8 = np.ascontiguousarray(
        PW8.reshape(B, PL, 2, 2, 128).transpose(0, 2, 4, 3, 1)
        .reshape(B, 2, 128, 2 * PL)
    )
    # pt16[b, h, p, c, i] = bf16(P*32)[b, h*512+i, c*128+p]
    P32b = (P * 32.0).astype(BF)
    pt16 = np.ascontiguousarray(
        P32b.reshape(B, 2, HW, ND, 128).transpose(0, 1, 4, 3, 2)
        .reshape(B, 2, 128, ND * HW)
    )
    pn8 = np.ascontiguousarray(
        P8.reshape(B, NQ, 2, 128, D).transpose(0, 3, 2, 1, 4).reshape(B, 128, 2 * NQ * D)
    )
    # ph16[b, h, p, ib4, e] = bf16(P/2)[b, (h*4+ib4)*128+p, e]
    ph16 = np.ascontiguousarray(
        (P * 0.5).astype(BF).reshape(B, 2, 4, 128, D).transpose(0, 1, 3, 2, 4)
        .reshape(B, 2, 128, 4 * D)
    )
    sjc = np.ascontiguousarray(
        (P @ wb).reshape(B, NI, 128).transpose(0, 2, 1)
    ).astype(np.float32)

    ws = [np.asarray(w, np.float32) for w in (w1, w2, w3)]
    # bf16 P-half weights for gates 0,1: slot g*4+c = w_g[c*128:(c+1)*128]*256
    wb16 = np.zeros((128, NB16, D), dtype=BF)
    for g in range(2):
        wb16[:, g * 4 : (g + 1) * 4] = (
            (ws[g][:D] * 256.0).astype(BF).reshape(ND, 128, D).transpose(1, 0, 2)
        )
    # fp8 slots: 0..5 attn-half pairs (g*2+kp, scale 128); 6,7 g2 P-half (scale 256)
    w8p = np.zeros((128, 2, N8, D), dtype=F8)
    for g in range(3):
        w8p[:, :, g * 2 : (g + 1) * 2] = (
            _q8(ws[g][D:] * 128.0).reshape(2, 2, 128, D).transpose(2, 1, 0, 3)
        )
    w8p[:, :, 6:8] = _q8(ws[2][:D] * 256.0).reshape(2, 2, 128, D).transpose(2, 1, 0, 3)

    biases = np.stack([np.asarray(b, np.float32) for b in (b1, b2, b3)])
    with_bias = bool(np.any(biases))

    base = {
        "wb16": wb16.reshape(128, NB16 * D),
        "w8p": w8p.reshape(128, 2 * N8 * D),
        "ones8": np.ones((128, 2 * 128), dtype=F8),
    }
    if with_bias:
        base["b32"] = biases
    in_maps = []
    for c in range(NCORES):
        sl = slice(c * BPC, (c + 1) * BPC)
        m = dict(base)
        m["pt8"] = pt8[sl]
        m["pwt8"] = pwt8[sl]
        m["pt16"] = pt16[sl]
        m["pn8"] = pn8[sl]
        m["ph16"] = ph16[sl]
        m["sjc"] = sjc[sl]
        in_maps.append(m)
    return in_maps, with_bias


def run(P, w_atten, w1, w2, w3, b1, b2, b3, trace=False):
    in_maps, with_bias = _prep_in_maps(P, w_atten, w1, w2, w3, b1, b2, b3)
    nc = _get_nc(with_bias)
    res = run_bass_kernel_spmd(
        nc, in_maps, core_ids=list(range(NCORES)), trace=trace
    )
    out = np.concatenate([res.results[c]["out"] for c in range(NCORES)], axis=0)
    return out, res


def kernel(P, w_atten, w1, w2, w3, b1, b2, b3):
    out, _ = run(P, w_atten, w1, w2, w3, b1, b2, b3)
    return out


# revision 32
# speedup vs baseline: 1.0013x; 1.0013x over previous
"""Fused attention-encoding kernel for Trainium2, 8-core batch-parallel SPMD.

Problem (per batch b of 16, p=1024 tokens, d=512 features):
    A[i,j] = wa.P_i + wb.P_j + (wc*P_i).P_j        (si = wa.P_i cancels in softmax)
    SA     = softmax_j(A)
    attn   = SA @ P
    Pc     = [P, attn]
    out    = sigmoid(Pc@w2) * P + sigmoid(Pc@w3) * tanh(Pc@w1)

Strategy: batch-parallel over 8 cores (2 batches/core). Large matmuls run in
fp8 (e4m3, max 240) with DoubleRow perf mode — two K=128 subtiles packed along
the free dim stream 2 rows/cycle, 2x the bf16 rate. Scores and attention
tolerate fp8 directly (softmax smoothing); the gate matmuls are
precision-critical in their P-half, so the z and r gates use bf16 there while
f and all attn-halves stay fp8. Everything accumulates in one fp32 PSUM group
via matched power-of-2 scales (P x32, w x256, attn^T x64, w_attn x128 — all
products 8192, descaled inside the activation).

Each batch is processed as two independent i-halves (softmax is over j, so
any i-range is self-contained): scores -> exp -> rowsum -> attn -> gates per
half. That gives a 4-unit pipeline per core whose emission interleaves unit
u's scores with unit u-1's gates, so the PE stays busy while the scalar
engine runs the exps, and it halves the warm-up bubble and drain tail.

sj = P@wb is computed host-side and folded into the exp as a per-partition
activation bias. The softmax denominator comes from an all-ones DoubleRow
matmul with M=128 so every PSUM partition holds the row sum (no partition
broadcast); sigmoid(x) is evaluated as (1+tanh(x/2))/2 so Exp and Tanh share
one activation-table set (no ACT_TABLE_LOAD thrash), with the affine fix-up
folded into the DVE combine against a host-shipped P/2. Input DMAs are spread
across the sync (~97GB/s), scalar (~87GB/s) and pool (~48GB/s) queues in
need-order, tiled per (k-pair, i-half) so whole-tile dependencies resolve as
early as possible.
"""

import sys

if "/opt/trn_rl_repo" not in sys.path:
    sys.path.insert(0, "/opt/trn_rl_repo")

from contextlib import ExitStack

import ml_dtypes
import numpy as np

import concourse.bass as bass
import concourse.mybir as mybir
import concourse.tile as tile
from concourse import bacc
from concourse.bass_utils import run_bass_kernel_spmd

B, PL, D = 16, 1024, 512
NCORES = 8
BPC = B // NCORES          # batches per core
NI = PL // 128             # token blocks: 8
NQ = NI // 2               # token block pairs: 4
ND = D // 128              # feature chunks: 4
HW = PL // 2               # i-half width: 512
FP32 = mybir.dt.float32
BF16 = mybir.dt.bfloat16
FP8 = mybir.dt.float8e4
AF = mybir.ActivationFunctionType
DR = mybir.MatmulPerfMode.DoubleRow
ALU = mybir.AluOpType
F8 = ml_dtypes.float8_e4m3   # IEEE-style e4m3, max 240 — matches TRN fp8e4
BF = ml_dtypes.bfloat16

DESCALE = 1.0 / 8192.0
NB16 = 8                   # bf16 weight slots: gates 0,1 x 4 chunks
N8 = 8                     # fp8 weight slots: 3 attn-half pairs x2, g2 P pairs x2

_cache = {}


def _build(with_bias: bool):
    nc = bacc.Bacc(
        "TRN2", target_bir_lowering=False, debug=False, num_devices=1
    )
    # pt8: per (k-pair, i-half) tiles; pt16/ph16: per i-half tiles
    pt8_d = nc.dram_tensor("pt8", [BPC, 2, 2, 128, 2 * HW], FP8, kind="ExternalInput").ap()
    pwt8_d = nc.dram_tensor("pwt8", [BPC, 2, 128, 2 * PL], FP8, kind="ExternalInput").ap()
    pt16_d = nc.dram_tensor("pt16", [BPC, 2, 128, ND * HW], BF16, kind="ExternalInput").ap()
    pn8_d = nc.dram_tensor("pn8", [BPC, 128, 2 * NQ * D], FP8, kind="ExternalInput").ap()
    ph16_d = nc.dram_tensor("ph16", [BPC, 2, 128, 4 * D], BF16, kind="ExternalInput").ap()
    sj_d = nc.dram_tensor("sjc", [BPC, 128, NI], FP32, kind="ExternalInput").ap()
    wb16_d = nc.dram_tensor("wb16", [128, NB16 * D], BF16, kind="ExternalInput").ap()
    w8_d = nc.dram_tensor("w8p", [128, 2 * N8 * D], FP8, kind="ExternalInput").ap()
    one_d = nc.dram_tensor("ones8", [128, 2 * 128], FP8, kind="ExternalInput").ap()
    if with_bias:
        b_d = nc.dram_tensor("b32", [3, D], FP32, kind="ExternalInput").ap()
    out_d = nc.dram_tensor("out", [BPC, PL, D], FP32, kind="ExternalOutput").ap()

    with tile.TileContext(nc) as tc, ExitStack() as ctx:
        pool = lambda name, bufs: ctx.enter_context(
            tc.tile_pool(name=name, bufs=bufs)
        )
        const = pool("const", 1)
        pt8p = pool("pt8", 2)
        pwt8p = pool("pwt8", 2)
        pt16p = pool("pt16", 2)
        pn8p = pool("pn8", 2)
        ph16p = pool("ph16", 2)
        sjp = pool("sj", 2)
        e8p = pool("e8", 2 * NQ)
        at8p = pool("at8", 4)
        rbbp = pool("rbb", 4)
        gactp = pool("gact", 2)
        combp = pool("comb", 2)
        # PSUM: pss 4x[128,512] (banks 0-3), psg 3x[128,512] (banks 4-6)
        pssp = ctx.enter_context(tc.tile_pool(name="pss", bufs=4, space="PSUM"))
        psgp = ctx.enter_context(tc.tile_pool(name="psg", bufs=3, space="PSUM"))

        wb16 = const.tile([128, NB16 * D], BF16, tag="wb16")
        w8p_t = const.tile([128, 2, N8 * D], FP8, tag="w8p")
        ones8 = const.tile([128, 2, 128], FP8, tag="ones8")
        if with_bias:
            bb = [
                const.tile([128, D], FP32, tag=f"bias{g}", name=f"bias{g}")
                for g in range(3)
            ]
            btmp = const.tile([1, 3 * D], FP32, tag="btmp")

        def load_weights():
            # wb16 rides sync (issued there after pn8); w8p + ones on scalar
            nc.sync.dma_start(wb16[:], wb16_d)
            nc.scalar.dma_start(w8p_t.rearrange("p t x -> p (t x)"), w8_d)
            nc.scalar.dma_start(ones8.rearrange("p t m -> p (t m)"), one_d)
            if with_bias:
                nc.scalar.dma_start(btmp[:], b_d.rearrange("g e -> (g e)")[None, :])
                for g in range(3):
                    nc.gpsimd.partition_broadcast(
                        bb[g][:], btmp[0:1, g * D : (g + 1) * D]
                    )

        def w8slot(s):
            return w8p_t[:, :, s * D : (s + 1) * D]

        st = {}  # per-batch/-unit tiles carried across the interleaved emission

        def emit_load(lb):
            # scalar ring: pt8 (kp,h) quarter-tiles then weight packs;
            # sync ring: pwt8 kp-halves, pn8, then per-half pt16/ph16 in
            # first-needed order; pool ring: sj (tiny) and batch-0 outputs
            pt8 = {}
            for h in range(2):
                for kp in range(2):
                    t = pt8p.tile([128, 2, HW], FP8, tag=f"pt8_{kp}{h}",
                                  name=f"pt8_{kp}{h}_{lb}")
                    nc.scalar.dma_start(
                        t.rearrange("p t x -> p (t x)"), pt8_d[lb, kp, h]
                    )
                    pt8[(kp, h)] = t
            pwt8 = []
            for kp in range(2):
                t = pwt8p.tile([128, 2, PL], FP8, tag=f"pwt8{kp}",
                               name=f"pwt8{kp}_{lb}")
                nc.sync.dma_start(t.rearrange("p t x -> p (t x)"), pwt8_d[lb, kp])
                pwt8.append(t)
            sj = sjp.tile([128, NI], FP32, tag="sj")
            nc.gpsimd.dma_start(sj[:], sj_d[lb])
            pn8 = pn8p.tile([128, 2, NQ * D], FP8, tag="pn8")
            nc.sync.dma_start(pn8.rearrange("p t x -> p (t x)"), pn8_d[lb])
            if lb == 0:
                load_weights()
            pt16, ph16 = [], []
            for h in range(2):
                t16 = pt16p.tile([128, ND * HW], BF16, tag=f"pt16{h}",
                                 name=f"pt16{h}_{lb}")
                nc.sync.dma_start(t16[:], pt16_d[lb, h])
                pt16.append(t16)
                p16 = ph16p.tile([128, 4 * D], BF16, tag=f"ph16{h}",
                                 name=f"ph16{h}_{lb}")
                nc.sync.dma_start(p16[:], ph16_d[lb, h])
                ph16.append(p16)
            st[lb] = dict(pt8=pt8, pwt8=pwt8, pt16=pt16, pn8=pn8, sj=sj,
                          ph16=ph16)

        def emit_scores(lb, h):
            s = st[lb]
            e8 = []
            for jb in range(NI):
                if jb % 2 == 0:
                    e8.append(
                        e8p.tile([128, 2, HW], FP8, tag="e8",
                                 name=f"e8_{lb}{h}_{jb}")
                    )
                ps_s = pssp.tile([128, HW], FP32, tag="pss",
                                 name=f"pss{lb}{h}_{jb}")
                for kp in range(2):
                    nc.tensor.matmul(
                        ps_s[:],
                        s["pwt8"][kp][:, :, jb * 128 : (jb + 1) * 128],
                        s["pt8"][(kp, h)][:],
                        start=(kp == 0),
                        stop=(kp == 1),
                        perf_mode=DR,
                    )
                nc.scalar.activation(
                    e8[jb // 2][:, jb % 2, :],
                    ps_s[:],
                    AF.Exp,
                    bias=s["sj"][:, jb : jb + 1],
                    scale=DESCALE,
                )
            st[(lb, h)] = dict(e8=e8)

        def emit_attn(lb, h):
            s, u = st[lb], st[(lb, h)]
            e8 = u["e8"]
            # rowsum over j via all-ones matmul with M=128: every PSUM
            # partition gets the sum, so no partition broadcast is needed
            rs = pssp.tile([128, HW], FP32, tag="pss", name=f"psrs{lb}{h}")
            for q in range(NQ):
                nc.tensor.matmul(
                    rs[:],
                    ones8[:],
                    e8[q][:],
                    start=(q == 0),
                    stop=(q == NQ - 1),
                    perf_mode=DR,
                )
            rbb = rbbp.tile([128, HW], FP32, tag="rbb", name=f"rbb{lb}{h}")
            nc.vector.reciprocal_approx_fast(out=rbb[:], in_=rs[:])
            # attn^T per d-chunk; normalize (x2 folds the 64/32 scale shift)
            at8 = at8p.tile([128, ND * HW], FP8, tag="at8", name=f"at8_{lb}{h}")
            pnv = s["pn8"]
            for dc in range(ND):
                ps_a = pssp.tile([128, HW], FP32, tag="pss",
                                 name=f"psa{lb}{h}_{dc}")
                for q in range(NQ):
                    nc.tensor.matmul(
                        ps_a[:],
                        pnv[:, :, q * D + dc * 128 : q * D + (dc + 1) * 128],
                        e8[q][:],
                        start=(q == 0),
                        stop=(q == NQ - 1),
                        perf_mode=DR,
                    )
                nc.vector.scalar_tensor_tensor(
                    out=at8[:, dc * HW : (dc + 1) * HW],
                    in0=ps_a[:],
                    scalar=2.0,
                    in1=rbb[:],
                    op0=ALU.mult,
                    op1=ALU.mult,
                )
            u["at8"] = at8

        def emit_gates(lb, h):
            s, u = st[lb], st[(lb, h)]
            atv = u["at8"].rearrange("p (c i) -> p c i", i=HW)
            pt16v = s["pt16"][h].rearrange("p (c i) -> p c i", i=HW)
            for ib4 in range(4):
                ib = h * 4 + ib4
                cols = slice(ib4 * 128, (ib4 + 1) * 128)
                ps_g = [
                    psgp.tile([128, 512], FP32, tag="psg",
                              name=f"psg{lb}{h}_{ib4}_{g}")
                    for g in range(3)
                ]
                for g in range(3):
                    # P-half: bf16 for z,r (precision), fp8 DR for f
                    if g < 2:
                        psteps = [
                            ("bf16", pt16v[:, c : c + 1, cols], g * 4 + c)
                            for c in range(ND)
                        ]
                    else:
                        psteps = [
                            ("fp8", s["pt8"][(kp, h)][:, :, cols], 6 + kp)
                            for kp in range(2)
                        ]
                    asteps = [
                        ("fp8", atv[:, 2 * kp : 2 * kp + 2, cols], g * 2 + kp)
                        for kp in range(2)
                    ]
                    steps = psteps + asteps
                    for si, (kind, lhsT, slot) in enumerate(steps):
                        if kind == "bf16":
                            nc.tensor.matmul(
                                ps_g[g],
                                lhsT,
                                wb16[:, slot * D : (slot + 1) * D],
                                start=(si == 0),
                                stop=(si == len(steps) - 1),
                            )
                        else:
                            nc.tensor.matmul(
                                ps_g[g],
                                lhsT,
                                w8slot(slot),
                                start=(si == 0),
                                stop=(si == len(steps) - 1),
                                perf_mode=DR,
                            )
                if with_bias:
                    for g in range(3):
                        nc.vector.tensor_add(ps_g[g][:], ps_g[g][:], bb[g][:])
                z32 = gactp.tile([128, D], FP32, tag="z32")
                r32 = gactp.tile([128, D], FP32, tag="r32")
                f32 = gactp.tile([128, D], FP32, tag="f32")
                nc.scalar.activation(z32[:], ps_g[0][:], AF.Tanh, scale=DESCALE)
                nc.scalar.activation(r32[:], ps_g[1][:], AF.Tanh, scale=DESCALE / 2)
                nc.scalar.activation(f32[:], ps_g[2][:], AF.Tanh, scale=DESCALE / 2)
                # out = (1+r')/2*P + (1+f')/2*z  with r'=tanh(gr/2), f'=tanh(gf/2)
                a32 = combp.tile([128, D], FP32, tag="a32")
                nc.vector.scalar_tensor_tensor(
                    out=a32[:], in0=r32[:], scalar=1.0,
                    in1=s["ph16"][h][:, ib4 * D : (ib4 + 1) * D],
                    op0=ALU.add, op1=ALU.mult,
                )
                b32 = combp.tile([128, D], FP32, tag="b32")
                nc.vector.scalar_tensor_tensor(
                    out=b32[:], in0=f32[:], scalar=1.0, in1=z32[:],
                    op0=ALU.add, op1=ALU.mult,
                )
                o32 = combp.tile([128, D], FP32, tag="o32")
                nc.vector.scalar_tensor_tensor(
                    out=o32[:], in0=b32[:], scalar=0.5, in1=a32[:],
                    op0=ALU.mult, op1=ALU.add,
                )
                # batch 0's outputs ride the otherwise-idle pool ring; the
                # final batch's stay on sync so the last store drains fast
                eng = nc.gpsimd if lb == 0 else nc.sync
                eng.dma_start(out_d[lb, ib * 128 : (ib + 1) * 128, :], o32[:])

        units = [(lb, h) for lb in range(BPC) for h in range(2)]
        for u, (lb, h) in enumerate(units):
            if h == 0:
                emit_load(lb)
            emit_scores(lb, h)
            if u > 0:
                emit_gates(*units[u - 1])
            emit_attn(lb, h)
        emit_gates(*units[-1])

    nc.compile()
    return nc


def _get_nc(with_bias: bool):
    if with_bias not in _cache:
        _cache[with_bias] = _build(with_bias)
    return _cache[with_bias]


def _q8(x):
    return np.clip(np.asarray(x, np.float32), -240.0, 240.0).astype(F8)


def _prep_in_maps(P, w_atten, w1, w2, w3, b1, b2, b3):
    P = np.ascontiguousarray(np.asarray(P, dtype=np.float32))
    w_atten = np.asarray(w_atten, dtype=np.float32)
    wb, wc = w_atten[D : 2 * D], w_atten[2 * D :]

    P8 = _q8(P * 32.0)                       # [B, PL, D] fp8

    # pt8[b, kp, h, p, c2, i] = P8[b, h*512+i, (2kp+c2)*128+p]
    pt8 = np.ascontiguousarray(
        P8.reshape(B, 2, HW, 2, 2, 128).transpose(0, 3, 1, 5, 4, 2)
        .reshape(B, 2, 2, 128, 2 * HW)
    )
    # pwt8[b, kp, p, c2, j] = fp8(P*wc*256)[b, j, (2kp+c2)*128+p]
    PW8 = _q8(P * wc * 256.0)
    pwt8 = np.ascontiguousarray(
        PW8.reshape(B, PL, 2, 2, 128).transpose(0, 2, 4, 3, 1)
        .reshape(B, 2, 128, 2 * PL)
    )
    # pt16[b, h, p, c, i] = bf16(P*32)[b, h*512+i, c*128+p]
    P32b = (P * 32.0).astype(BF)
    pt16 = np.ascontiguousarray(
        P32b.reshape(B, 2, HW, ND, 128).transpose(0, 1, 4, 3, 2)
        .reshape(B, 2, 128, ND * HW)
    )
    pn8 = np.ascontiguousarray(
        P8.reshape(B, NQ, 2, 128, D).transpose(0, 3, 2, 1, 4).reshape(B, 128, 2 * NQ * D)
    )
    # ph16[b, h, p, ib4, e] = bf16(P/2)[b, (h*4+ib4)*128+p, e]
    ph16 = np.ascontiguousarray(
        (P * 0.5).astype(BF).reshape(B, 2, 4, 128, D).transpose(0, 1, 3, 2, 4)
        .reshape(B, 2, 128, 4 * D)
    )
    sjc = np.ascontiguousarray(
        (P @ wb).reshape(B, NI, 128).transpose(0, 2, 1)
    ).astype(np.float32)

    ws = [np.asarray(w, np.float32) for w in (w1, w2, w3)]
    # bf16 P-half weights for gates 0,1: slot g*4+c = w_g[c*128:(c+1)*128]*256
    wb16 = np.zeros((128, NB16, D), dtype=BF)
    for g in range(2):
        wb16[:, g * 4 : (g + 1) * 4] = (
            (ws[g][:D] * 256.0).astype(BF).reshape(ND, 128, D).transpose(1, 0, 2)
        )
    # fp8 slots: 0..5 attn-half pairs (g*2+kp, scale 128); 6,7 g2 P-half (scale 256)
    w8p = np.zeros((128, 2, N8, D), dtype=F8)
    for g in range(3):
        w8p[:, :, g * 2 : (g + 1) * 2] = (
            _q8(ws[g][D:] * 128.0).reshape(2, 2, 128, D).transpose(2, 1, 0, 3)
        )
    w8p[:, :, 6:8] = _q8(ws[2][:D] * 256.0).reshape(2, 2, 128, D).transpose(2, 1, 0, 3)

    biases = np.stack([np.asarray(b, np.float32) for b in (b1, b2, b3)])
    with_bias = bool(np.any(biases))

    base = {
        "wb16": wb16.reshape(128, NB16 * D),
        "w8p": w8p.reshape(128, 2 * N8 * D),
        "ones8": np.ones((128, 2 * 128), dtype=F8),
    }
    if with_bias:
        base["b32"] = biases
    in_maps = []
    for c in range(NCORES):
        sl = slice(c * BPC, (c + 1) * BPC)
        m = dict(base)
        m["pt8"] = pt8[sl]
        m["pwt8"] = pwt8[sl]
        m["pt16"] = pt16[sl]
        m["pn8"] = pn8[sl]
        m["ph16"] = ph16[sl]
        m["sjc"] = sjc[sl]
        in_maps.append(m)
    return in_maps, with_bias


def run(P, w_atten, w1, w2, w3, b1, b2, b3, trace=False):
    in_maps, with_bias = _prep_in_maps(P, w_atten, w1, w2, w3, b1, b2, b3)
    nc = _get_nc(with_bias)
    res = run_bass_kernel_spmd(
        nc, in_maps, core_ids=list(range(NCORES)), trace=trace
    )
    out = np.concatenate([res.results[c]["out"] for c in range(NCORES)], axis=0)
    return out, res


def kernel(P, w_atten, w1, w2, w3, b1, b2, b3):
    out, _ = run(P, w_atten, w1, w2, w3, b1, b2, b3)
    return out


# revision 33
# speedup vs baseline: 1.0473x; 1.0459x over previous
"""Fused attention-encoding kernel for Trainium2, 8-core batch-parallel SPMD.

Problem (per batch b of 16, p=1024 tokens, d=512 features):
    A[i,j] = wa.P_i + wb.P_j + (wc*P_i).P_j        (si = wa.P_i cancels in softmax)
    SA     = softmax_j(A)
    attn   = SA @ P
    Pc     = [P, attn]
    out    = sigmoid(Pc@w2) * P + sigmoid(Pc@w3) * tanh(Pc@w1)

Strategy: batch-parallel over 8 cores (2 batches/core). Large matmuls run in
fp8 (e4m3, max 240) with DoubleRow perf mode — two K=128 subtiles packed along
the free dim stream 2 rows/cycle, 2x the bf16 rate. Scores and attention
tolerate fp8 directly (softmax smoothing); the gate matmuls are
precision-critical in their P-half, so the z and r gates use bf16 there while
f and all attn-halves stay fp8. Everything accumulates in one fp32 PSUM group
via matched power-of-2 scales (P x32, w x256, attn^T x64, w_attn x128 — all
products 8192, descaled inside the activation).

Each batch is processed as two independent i-halves (softmax is over j, so
any i-range is self-contained): scores -> exp -> rowsum -> attn -> gates per
half. That gives a 4-unit pipeline per core whose emission interleaves unit
u's scores with unit u-1's gates, so the PE stays busy while the scalar
engine runs the exps, and it halves the warm-up bubble and drain tail.

sj = P@wb is computed host-side and folded into the exp as a per-partition
activation bias. The softmax denominator comes from an all-ones DoubleRow
matmul with M=128 so every PSUM partition holds the row sum (no partition
broadcast); sigmoid(x) is evaluated as (1+tanh(x/2))/2 so Exp and Tanh share
one activation-table set (no ACT_TABLE_LOAD thrash), with the affine fix-up
folded into the DVE combine against a host-shipped P/2. Input DMAs are spread
across the sync (~97GB/s), scalar (~87GB/s) and pool (~48GB/s) queues in
need-order, tiled per (k-pair, i-half) so whole-tile dependencies resolve as
early as possible.
"""

import sys

if "/opt/trn_rl_repo" not in sys.path:
    sys.path.insert(0, "/opt/trn_rl_repo")

from contextlib import ExitStack

import ml_dtypes
import numpy as np

import concourse.bass as bass
import concourse.mybir as mybir
import concourse.tile as tile
from concourse import bacc
from concourse.bass_utils import run_bass_kernel_spmd

B, PL, D = 16, 1024, 512
NCORES = 8
BPC = B // NCORES          # batches per core
NI = PL // 128             # token blocks: 8
NQ = NI // 2               # token block pairs: 4
ND = D // 128              # feature chunks: 4
HW = PL // 2               # i-half width: 512
FP32 = mybir.dt.float32
BF16 = mybir.dt.bfloat16
FP8 = mybir.dt.float8e4
AF = mybir.ActivationFunctionType
DR = mybir.MatmulPerfMode.DoubleRow
ALU = mybir.AluOpType
F8 = ml_dtypes.float8_e4m3   # IEEE-style e4m3, max 240 — matches TRN fp8e4
BF = ml_dtypes.bfloat16

DESCALE = 1.0 / 8192.0
NB16 = 8                   # bf16 weight slots: gates 0,1 x 4 chunks
N8 = 8                     # fp8 weight slots: 3 attn-half pairs x2, g2 P pairs x2

_cache = {}


def _build(with_bias: bool):
    nc = bacc.Bacc(
        "TRN2", target_bir_lowering=False, debug=False, num_devices=1
    )
    # pt8: per (k-pair, i-half) tiles; pt16/ph16: per i-half tiles
    pt8_d = nc.dram_tensor("pt8", [BPC, 2, 2, 128, 2 * HW], FP8, kind="ExternalInput").ap()
    pwt8_d = nc.dram_tensor("pwt8", [BPC, 2, 128, 2 * PL], FP8, kind="ExternalInput").ap()
    pt16_d = nc.dram_tensor("pt16", [BPC, 2, 128, ND * HW], BF16, kind="ExternalInput").ap()
    pn8_d = nc.dram_tensor("pn8", [BPC, 128, 2 * NQ * D], FP8, kind="ExternalInput").ap()
    ph16_d = nc.dram_tensor("ph16", [BPC, 2, 128, 4 * D], BF16, kind="ExternalInput").ap()
    sj_d = nc.dram_tensor("sjc", [BPC, 128, NI], FP32, kind="ExternalInput").ap()
    wb16_d = nc.dram_tensor("wb16", [128, NB16 * D], BF16, kind="ExternalInput").ap()
    w8_d = nc.dram_tensor("w8p", [128, 2 * N8 * D], FP8, kind="ExternalInput").ap()
    one_d = nc.dram_tensor("ones8", [128, 2 * 128], FP8, kind="ExternalInput").ap()
    if with_bias:
        b_d = nc.dram_tensor("b32", [3, D], FP32, kind="ExternalInput").ap()
    out_d = nc.dram_tensor("out", [BPC, PL, D], FP32, kind="ExternalOutput").ap()

    with tile.TileContext(nc) as tc, ExitStack() as ctx:
        pool = lambda name, bufs: ctx.enter_context(
            tc.tile_pool(name=name, bufs=bufs)
        )
        const = pool("const", 1)
        pt8p = pool("pt8", 2)
        pwt8p = pool("pwt8", 2)
        pt16p = pool("pt16", 2)
        pn8p = pool("pn8", 2)
        ph16p = pool("ph16", 2)
        sjp = pool("sj", 2)
        e8p = pool("e8", 2 * NQ)
        at8p = pool("at8", 4)
        rbbp = pool("rbb", 4)
        gactp = pool("gact", 2)
        combp = pool("comb", 2)
        # PSUM: pss 4x[128,512] (banks 0-3), psg 3x[128,512] (banks 4-6)
        pssp = ctx.enter_context(tc.tile_pool(name="pss", bufs=4, space="PSUM"))
        psgp = ctx.enter_context(tc.tile_pool(name="psg", bufs=3, space="PSUM"))

        wb16 = const.tile([128, NB16 * D], BF16, tag="wb16")
        w8p_t = const.tile([128, 2, N8 * D], FP8, tag="w8p")
        ones8 = const.tile([128, 2, 128], FP8, tag="ones8")
        if with_bias:
            bb = [
                const.tile([128, D], FP32, tag=f"bias{g}", name=f"bias{g}")
                for g in range(3)
            ]
            btmp = const.tile([1, 3 * D], FP32, tag="btmp")

        def load_weights():
            nc.scalar.dma_start(wb16[:], wb16_d)
            nc.scalar.dma_start(w8p_t.rearrange("p t x -> p (t x)"), w8_d)
            nc.scalar.dma_start(ones8.rearrange("p t m -> p (t m)"), one_d)
            if with_bias:
                nc.scalar.dma_start(btmp[:], b_d.rearrange("g e -> (g e)")[None, :])
                for g in range(3):
                    nc.gpsimd.partition_broadcast(
                        bb[g][:], btmp[0:1, g * D : (g + 1) * D]
                    )

        def w8slot(s):
            return w8p_t[:, :, s * D : (s + 1) * D]

        st = {}  # per-batch/-unit tiles carried across the interleaved emission

        def emit_load(lb):
            # scalar ring: pt8 (kp,h) quarter-tiles then weight packs;
            # sync ring: pwt8 kp-halves, pn8, then per-half pt16/ph16 in
            # first-needed order; pool ring: sj (tiny) and batch-0 outputs
            pt8 = {}
            for h in range(2):
                for kp in range(2):
                    t = pt8p.tile([128, 2, HW], FP8, tag=f"pt8_{kp}{h}",
                                  name=f"pt8_{kp}{h}_{lb}")
                    nc.scalar.dma_start(
                        t.rearrange("p t x -> p (t x)"), pt8_d[lb, kp, h]
                    )
                    pt8[(kp, h)] = t
            pwt8 = []
            for kp in range(2):
                t = pwt8p.tile([128, 2, PL], FP8, tag=f"pwt8{kp}",
                               name=f"pwt8{kp}_{lb}")
                nc.sync.dma_start(t.rearrange("p t x -> p (t x)"), pwt8_d[lb, kp])
                pwt8.append(t)
            sj = sjp.tile([128, NI], FP32, tag="sj")
            nc.gpsimd.dma_start(sj[:], sj_d[lb])
            pn8 = pn8p.tile([128, 2, NQ * D], FP8, tag="pn8")
            nc.sync.dma_start(pn8.rearrange("p t x -> p (t x)"), pn8_d[lb])
            pt16, ph16 = [], []
            for h in range(2):
                t16 = pt16p.tile([128, ND * HW], BF16, tag=f"pt16{h}",
                                 name=f"pt16{h}_{lb}")
                nc.sync.dma_start(t16[:], pt16_d[lb, h])
                pt16.append(t16)
                p16 = ph16p.tile([128, 4 * D], BF16, tag=f"ph16{h}",
                                 name=f"ph16{h}_{lb}")
                nc.sync.dma_start(p16[:], ph16_d[lb, h])
                ph16.append(p16)
            if lb == 0:
                load_weights()
            st[lb] = dict(pt8=pt8, pwt8=pwt8, pt16=pt16, pn8=pn8, sj=sj,
                          ph16=ph16)

        def emit_scores(lb, h):
            s = st[lb]
            e8 = []
            for jb in range(NI):
                if jb % 2 == 0:
                    e8.append(
                        e8p.tile([128, 2, HW], FP8, tag="e8",
                                 name=f"e8_{lb}{h}_{jb}")
                    )
                ps_s = pssp.tile([128, HW], FP32, tag="pss",
                                 name=f"pss{lb}{h}_{jb}")
                for kp in range(2):
                    nc.tensor.matmul(
                        ps_s[:],
                        s["pwt8"][kp][:, :, jb * 128 : (jb + 1) * 128],
                        s["pt8"][(kp, h)][:],
                        start=(kp == 0),
                        stop=(kp == 1),
                        perf_mode=DR,
                    )
                nc.scalar.activation(
                    e8[jb // 2][:, jb % 2, :],
                    ps_s[:],
                    AF.Exp,
                    bias=s["sj"][:, jb : jb + 1],
                    scale=DESCALE,
                )
            st[(lb, h)] = dict(e8=e8)

        def emit_attn(lb, h):
            s, u = st[lb], st[(lb, h)]
            e8 = u["e8"]
            # rowsum over j via all-ones matmul with M=128: every PSUM
            # partition gets the sum, so no partition broadcast is needed
            rs = pssp.tile([128, HW], FP32, tag="pss", name=f"psrs{lb}{h}")
            for q in range(NQ):
                nc.tensor.matmul(
                    rs[:],
                    ones8[:],
                    e8[q][:],
                    start=(q == 0),
                    stop=(q == NQ - 1),
                    perf_mode=DR,
                )
            rbb = rbbp.tile([128, HW], FP32, tag="rbb", name=f"rbb{lb}{h}")
            nc.vector.reciprocal_approx_fast(out=rbb[:], in_=rs[:])
            # attn^T per d-chunk; normalize (x2 folds the 64/32 scale shift)
            at8 = at8p.tile([128, ND * HW], FP8, tag="at8", name=f"at8_{lb}{h}")
            pnv = s["pn8"]
            for dc in range(ND):
                ps_a = pssp.tile([128, HW], FP32, tag="pss",
                                 name=f"psa{lb}{h}_{dc}")
                for q in range(NQ):
                    nc.tensor.matmul(
                        ps_a[:],
                        pnv[:, :, q * D + dc * 128 : q * D + (dc + 1) * 128],
                        e8[q][:],
                        start=(q == 0),
                        stop=(q == NQ - 1),
                        perf_mode=DR,
                    )
                nc.vector.scalar_tensor_tensor(
                    out=at8[:, dc * HW : (dc + 1) * HW],
                    in0=ps_a[:],
                    scalar=2.0,
                    in1=rbb[:],
                    op0=ALU.mult,
                    op1=ALU.mult,
                )
            u["at8"] = at8

        def emit_gates(lb, h):
            s, u = st[lb], st[(lb, h)]
            atv = u["at8"].rearrange("p (c i) -> p c i", i=HW)
            pt16v = s["pt16"][h].rearrange("p (c i) -> p c i", i=HW)
            for ib4 in range(4):
                ib = h * 4 + ib4
                cols = slice(ib4 * 128, (ib4 + 1) * 128)
                ps_g = [
                    psgp.tile([128, 512], FP32, tag="psg",
                              name=f"psg{lb}{h}_{ib4}_{g}")
                    for g in range(3)
                ]
                for g in range(3):
                    # P-half: bf16 for z,r (precision), fp8 DR for f
                    if g < 2:
                        psteps = [
                            ("bf16", pt16v[:, c : c + 1, cols], g * 4 + c)
                            for c in range(ND)
                        ]
                    else:
                        psteps = [
                            ("fp8", s["pt8"][(kp, h)][:, :, cols], 6 + kp)
                            for kp in range(2)
                        ]
                    asteps = [
                        ("fp8", atv[:, 2 * kp : 2 * kp + 2, cols], g * 2 + kp)
                        for kp in range(2)
                    ]
                    steps = psteps + asteps
                    for si, (kind, lhsT, slot) in enumerate(steps):
                        if kind == "bf16":
                            nc.tensor.matmul(
                                ps_g[g],
                                lhsT,
                                wb16[:, slot * D : (slot + 1) * D],
                                start=(si == 0),
                                stop=(si == len(steps) - 1),
                            )
                        else:
                            nc.tensor.matmul(
                                ps_g[g],
                                lhsT,
                                w8slot(slot),
                                start=(si == 0),
                                stop=(si == len(steps) - 1),
                                perf_mode=DR,
                            )
                if with_bias:
                    for g in range(3):
                        nc.vector.tensor_add(ps_g[g][:], ps_g[g][:], bb[g][:])
                z32 = gactp.tile([128, D], FP32, tag="z32")
                r32 = gactp.tile([128, D], FP32, tag="r32")
                f32 = gactp.tile([128, D], FP32, tag="f32")
                nc.scalar.activation(z32[:], ps_g[0][:], AF.Tanh, scale=DESCALE)
                nc.scalar.activation(r32[:], ps_g[1][:], AF.Tanh, scale=DESCALE / 2)
                nc.scalar.activation(f32[:], ps_g[2][:], AF.Tanh, scale=DESCALE / 2)
                # out = (1+r')/2*P + (1+f')/2*z  with r'=tanh(gr/2), f'=tanh(gf/2)
                a32 = combp.tile([128, D], FP32, tag="a32")
                nc.vector.scalar_tensor_tensor(
                    out=a32[:], in0=r32[:], scalar=1.0,
                    in1=s["ph16"][h][:, ib4 * D : (ib4 + 1) * D],
                    op0=ALU.add, op1=ALU.mult,
                )
                b32 = combp.tile([128, D], FP32, tag="b32")
                nc.vector.scalar_tensor_tensor(
                    out=b32[:], in0=f32[:], scalar=1.0, in1=z32[:],
                    op0=ALU.add, op1=ALU.mult,
                )
                o32 = combp.tile([128, D], FP32, tag="o32")
                nc.vector.scalar_tensor_tensor(
                    out=o32[:], in0=b32[:], scalar=0.5, in1=a32[:],
                    op0=ALU.mult, op1=ALU.add,
                )
                # batch 0's outputs ride the otherwise-idle pool ring; the
                # final batch's stay on sync so the last store drains fast
                eng = nc.gpsimd if lb == 0 else nc.sync
                eng.dma_start(out_d[lb, ib * 128 : (ib + 1) * 128, :], o32[:])

        units = [(lb, h) for lb in range(BPC) for h in range(2)]
        for u, (lb, h) in enumerate(units):
            if h == 0:
                emit_load(lb)
            emit_scores(lb, h)
            if u > 0:
                emit_gates(*units[u - 1])
            emit_attn(lb, h)
        emit_gates(*units[-1])

    nc.compile()
    return nc


def _get_nc(with_bias: bool):
    if with_bias not in _cache:
        _cache[with_bias] = _build(with_bias)
    return _cache[with_bias]


def _q8(x):
    return np.clip(np.asarray(x, np.float32), -240.0, 240.0).astype(F8)


def _prep_in_maps(P, w_atten, w1, w2, w3, b1, b2, b3):
    P = np.ascontiguousarray(np.asarray(P, dtype=np.float32))
    w_atten = np.asarray(w_atten, dtype=np.float32)
    wb, wc = w_atten[D : 2 * D], w_atten[2 * D :]

    P8 = _q8(P * 32.0)                       # [B, PL, D] fp8

    # pt8[b, kp, h, p, c2, i] = P8[b, h*512+i, (2kp+c2)*128+p]
    pt8 = np.ascontiguousarray(
        P8.reshape(B, 2, HW, 2, 2, 128).transpose(0, 3, 1, 5, 4, 2)
        .reshape(B, 2, 2, 128, 2 * HW)
    )
    # pwt8[b, kp, p, c2, j] = fp8(P*wc*256)[b, j, (2kp+c2)*128+p]
    PW8 = _q8(P * wc * 256.0)
    pwt8 = np.ascontiguousarray(
        PW8.reshape(B, PL, 2, 2, 128).transpose(0, 2, 4, 3, 1)
        .reshape(B, 2, 128, 2 * PL)
    )
    # pt16[b, h, p, c, i] = bf16(P*32)[b, h*512+i, c*128+p]
    P32b = (P * 32.0).astype(BF)
    pt16 = np.ascontiguousarray(
        P32b.reshape(B, 2, HW, ND, 128).transpose(0, 1, 4, 3, 2)
        .reshape(B, 2, 128, ND * HW)
    )
    pn8 = np.ascontiguousarray(
        P8.reshape(B, NQ, 2, 128, D).transpose(0, 3, 2, 1, 4).reshape(B, 128, 2 * NQ * D)
    )
    # ph16[b, h, p, ib4, e] = bf16(P/2)[b, (h*4+ib4)*128+p, e]
    ph16 = np.ascontiguousarray(
        (P * 0.5).astype(BF).reshape(B, 2, 4, 128, D).transpose(0, 1, 3, 2, 4)
        .reshape(B, 2, 128, 4 * D)
    )
    sjc = np.ascontiguousarray(
        (P @ wb).reshape(B, NI, 128).transpose(0, 2, 1)
    ).astype(np.float32)

    ws = [np.asarray(w, np.float32) for w in (w1, w2, w3)]
    # bf16 P-half weights for gates 0,1: slot g*4+c = w_g[c*128:(c+1)*128]*256
    wb16 = np.zeros((128, NB16, D), dtype=BF)
    for g in range(2):
        wb16[:, g * 4 : (g + 1) * 4] = (
            (ws[g][:D] * 256.0).astype(BF).reshape(ND, 128, D).transpose(1, 0, 2)
        )
    # fp8 slots: 0..5 attn-half pairs (g*2+kp, scale 128); 6,7 g2 P-half (scale 256)
    w8p = np.zeros((128, 2, N8, D), dtype=F8)
    for g in range(3):
        w8p[:, :, g * 2 : (g + 1) * 2] = (
            _q8(ws[g][D:] * 128.0).reshape(2, 2, 128, D).transpose(2, 1, 0, 3)
        )
    w8p[:, :, 6:8] = _q8(ws[2][:D] * 256.0).reshape(2, 2, 128, D).transpose(2, 1, 0, 3)

    biases = np.stack([np.asarray(b, np.float32) for b in (b1, b2, b3)])
    with_bias = bool(np.any(biases))

    base = {
        "wb16": wb16.reshape(128, NB16 * D),
        "w8p": w8p.reshape(128, 2 * N8 * D),
        "ones8": np.ones((128, 2 * 128), dtype=F8),
    }
    if with_bias:
        base["b32"] = biases
    in_maps = []
    for c in range(NCORES):
        sl = slice(c * BPC, (c + 1) * BPC)
        m = dict(base)
        m["pt8"] = pt8[sl]
        m["pwt8"] = pwt8[sl]
        m["pt16"] = pt16[sl]
        m["pn8"] = pn8[sl]
        m["ph16"] = ph16[sl]
        m["sjc"] = sjc[sl]
        in_maps.append(m)
    return in_maps, with_bias


def run(P, w_atten, w1, w2, w3, b1, b2, b3, trace=False):
    in_maps, with_bias = _prep_in_maps(P, w_atten, w1, w2, w3, b1, b2, b3)
    nc = _get_nc(with_bias)
    res = run_bass_kernel_spmd(
        nc, in_maps, core_ids=list(range(NCORES)), trace=trace
    )
    out = np.concatenate([res.results[c]["out"] for c in range(NCORES)], axis=0)
    return out, res


def kernel(P, w_atten, w1, w2, w3, b1, b2, b3):
    out, _ = run(P, w_atten, w1, w2, w3, b1, b2, b3)
    return out


# revision 34
# speedup vs baseline: 1.0515x; 1.0040x over previous
"""Fused attention-encoding kernel for Trainium2, 8-core batch-parallel SPMD.

Problem (per batch b of 16, p=1024 tokens, d=512 features):
    A[i,j] = wa.P_i + wb.P_j + (wc*P_i).P_j        (si = wa.P_i cancels in softmax)
    SA     = softmax_j(A)
    attn   = SA @ P
    Pc     = [P, attn]
    out    = sigmoid(Pc@w2) * P + sigmoid(Pc@w3) * tanh(Pc@w1)

Strategy: batch-parallel over 8 cores (2 batches/core). Large matmuls run in
fp8 (e4m3, max 240) with DoubleRow perf mode — two K=128 subtiles packed along
the free dim stream 2 rows/cycle, 2x the bf16 rate. Scores and attention
tolerate fp8 directly (softmax smoothing); the gate matmuls are
precision-critical in their P-half, so the z and r gates use bf16 there while
f and all attn-halves stay fp8. Everything accumulates in one fp32 PSUM group
via matched power-of-2 scales (P x32, w x256, attn^T x64, w_attn x128 — all
products 8192, descaled inside the activation).

Each batch is processed as two independent i-halves (softmax is over j, so
any i-range is self-contained): scores -> exp -> rowsum -> attn -> gates per
half. That gives a 4-unit pipeline per core whose emission interleaves unit
u's scores with unit u-1's gates, so the PE stays busy while the scalar
engine runs the exps, and it halves the warm-up bubble and drain tail.

sj = P@wb is computed host-side and folded into the exp as a per-partition
activation bias. The softmax denominator comes from an all-ones DoubleRow
matmul with M=128 so every PSUM partition holds the row sum (no partition
broadcast); sigmoid(x) is evaluated as (1+tanh(x/2))/2 so Exp and Tanh share
one activation-table set (no ACT_TABLE_LOAD thrash), with the affine fix-up
folded into the DVE combine against a host-shipped P/2. Input DMAs are spread
across the sync (~97GB/s), scalar (~87GB/s) and pool (~48GB/s) queues in
need-order, tiled per (k-pair, i-half) so whole-tile dependencies resolve as
early as possible.
"""

import sys

if "/opt/trn_rl_repo" not in sys.path:
    sys.path.insert(0, "/opt/trn_rl_repo")

from contextlib import ExitStack

import ml_dtypes
import numpy as np

import concourse.bass as bass
import concourse.mybir as mybir
import concourse.tile as tile
from concourse import bacc
from concourse.bass_utils import run_bass_kernel_spmd

B, PL, D = 16, 1024, 512
NCORES = 8
BPC = B // NCORES          # batches per core
NI = PL // 128             # token blocks: 8
NQ = NI // 2               # token block pairs: 4
ND = D // 128              # feature chunks: 4
HW = PL // 2               # i-half width: 512
FP32 = mybir.dt.float32
BF16 = mybir.dt.bfloat16
FP8 = mybir.dt.float8e4
AF = mybir.ActivationFunctionType
DR = mybir.MatmulPerfMode.DoubleRow
ALU = mybir.AluOpType
F8 = ml_dtypes.float8_e4m3   # IEEE-style e4m3, max 240 — matches TRN fp8e4
BF = ml_dtypes.bfloat16

DESCALE = 1.0 / 8192.0
NB16 = 8                   # bf16 weight slots: gates 0,1 x 4 chunks
N8 = 8                     # fp8 weight slots: 3 attn-half pairs x2, g2 P pairs x2

_cache = {}


def _build(with_bias: bool):
    nc = bacc.Bacc(
        "TRN2", target_bir_lowering=False, debug=False, num_devices=1
    )
    # pt8: per (k-pair, i-half) tiles; pt16/ph16: per i-half tiles
    pt8_d = nc.dram_tensor("pt8", [BPC, 2, 2, 128, 2 * HW], FP8, kind="ExternalInput").ap()
    pwt8_d = nc.dram_tensor("pwt8", [BPC, 2, 128, 2 * PL], FP8, kind="ExternalInput").ap()
    pt16_d = nc.dram_tensor("pt16", [BPC, 2, 128, ND * HW], BF16, kind="ExternalInput").ap()
    pn8_d = nc.dram_tensor("pn8", [BPC, 128, 2 * NQ * D], FP8, kind="ExternalInput").ap()
    ph16_d = nc.dram_tensor("ph16", [BPC, 2, 128, 4 * D], BF16, kind="ExternalInput").ap()
    sj_d = nc.dram_tensor("sjc", [BPC, 128, NI], FP32, kind="ExternalInput").ap()
    wb16_d = nc.dram_tensor("wb16", [128, NB16 * D], BF16, kind="ExternalInput").ap()
    w8_d = nc.dram_tensor("w8p", [128, 2 * N8 * D], FP8, kind="ExternalInput").ap()
    one_d = nc.dram_tensor("ones8", [128, 2 * 128], FP8, kind="ExternalInput").ap()
    if with_bias:
        b_d = nc.dram_tensor("b32", [3, D], FP32, kind="ExternalInput").ap()
    out_d = nc.dram_tensor("out", [BPC, PL, D], FP32, kind="ExternalOutput").ap()

    with tile.TileContext(nc) as tc, ExitStack() as ctx:
        pool = lambda name, bufs: ctx.enter_context(
            tc.tile_pool(name=name, bufs=bufs)
        )
        const = pool("const", 1)
        pt8p = pool("pt8", 2)
        pwt8p = pool("pwt8", 2)
        pt16p = pool("pt16", 2)
        pn8p = pool("pn8", 2)
        ph16p = pool("ph16", 2)
        sjp = pool("sj", 2)
        e8p = pool("e8", 2 * NQ)
        at8p = pool("at8", 4)
        rbbp = pool("rbb", 4)
        gactp = pool("gact", 2)
        combp = pool("comb", 2)
        # PSUM: pss 4x[128,512] (banks 0-3), psg 3x[128,512] (banks 4-6)
        pssp = ctx.enter_context(tc.tile_pool(name="pss", bufs=5, space="PSUM"))
        psgp = ctx.enter_context(tc.tile_pool(name="psg", bufs=3, space="PSUM"))

        wb16 = const.tile([128, NB16 * D], BF16, tag="wb16")
        w8p_t = const.tile([128, 2, N8 * D], FP8, tag="w8p")
        ones8 = const.tile([128, 2, 128], FP8, tag="ones8")
        if with_bias:
            bb = [
                const.tile([128, D], FP32, tag=f"bias{g}", name=f"bias{g}")
                for g in range(3)
            ]
            btmp = const.tile([1, 3 * D], FP32, tag="btmp")

        def load_weights():
            nc.scalar.dma_start(wb16[:], wb16_d)
            nc.scalar.dma_start(w8p_t.rearrange("p t x -> p (t x)"), w8_d)
            nc.scalar.dma_start(ones8.rearrange("p t m -> p (t m)"), one_d)
            if with_bias:
                nc.scalar.dma_start(btmp[:], b_d.rearrange("g e -> (g e)")[None, :])
                for g in range(3):
                    nc.gpsimd.partition_broadcast(
                        bb[g][:], btmp[0:1, g * D : (g + 1) * D]
                    )

        def w8slot(s):
            return w8p_t[:, :, s * D : (s + 1) * D]

        st = {}  # per-batch/-unit tiles carried across the interleaved emission

        def emit_load(lb):
            # scalar ring: pt8 (kp,h) quarter-tiles then weight packs;
            # sync ring: pwt8 kp-halves, pn8, then per-half pt16/ph16 in
            # first-needed order; pool ring: sj (tiny) and batch-0 outputs
            pt8 = {}
            for h in range(2):
                for kp in range(2):
                    t = pt8p.tile([128, 2, HW], FP8, tag=f"pt8_{kp}{h}",
                                  name=f"pt8_{kp}{h}_{lb}")
                    nc.scalar.dma_start(
                        t.rearrange("p t x -> p (t x)"), pt8_d[lb, kp, h]
                    )
                    pt8[(kp, h)] = t
            pwt8 = []
            for kp in range(2):
                t = pwt8p.tile([128, 2, PL], FP8, tag=f"pwt8{kp}",
                               name=f"pwt8{kp}_{lb}")
                nc.sync.dma_start(t.rearrange("p t x -> p (t x)"), pwt8_d[lb, kp])
                pwt8.append(t)
            sj = sjp.tile([128, NI], FP32, tag="sj")
            nc.gpsimd.dma_start(sj[:], sj_d[lb])
            pn8 = pn8p.tile([128, 2, NQ * D], FP8, tag="pn8")
            nc.sync.dma_start(pn8.rearrange("p t x -> p (t x)"), pn8_d[lb])
            pt16, ph16 = [], []
            for h in range(2):
                t16 = pt16p.tile([128, ND * HW], BF16, tag=f"pt16{h}",
                                 name=f"pt16{h}_{lb}")
                nc.sync.dma_start(t16[:], pt16_d[lb, h])
                pt16.append(t16)
                p16 = ph16p.tile([128, 4 * D], BF16, tag=f"ph16{h}",
                                 name=f"ph16{h}_{lb}")
                nc.sync.dma_start(p16[:], ph16_d[lb, h])
                ph16.append(p16)
            if lb == 0:
                load_weights()
            st[lb] = dict(pt8=pt8, pwt8=pwt8, pt16=pt16, pn8=pn8, sj=sj,
                          ph16=ph16)

        def emit_scores(lb, h):
            s = st[lb]
            e8 = []
            for jb in range(NI):
                if jb % 2 == 0:
                    e8.append(
                        e8p.tile([128, 2, HW], FP8, tag="e8",
                                 name=f"e8_{lb}{h}_{jb}")
                    )
                ps_s = pssp.tile([128, HW], FP32, tag="pss",
                                 name=f"pss{lb}{h}_{jb}")
                for kp in range(2):
                    nc.tensor.matmul(
                        ps_s[:],
                        s["pwt8"][kp][:, :, jb * 128 : (jb + 1) * 128],
                        s["pt8"][(kp, h)][:],
                        start=(kp == 0),
                        stop=(kp == 1),
                        perf_mode=DR,
                    )
                nc.scalar.activation(
                    e8[jb // 2][:, jb % 2, :],
                    ps_s[:],
                    AF.Exp,
                    bias=s["sj"][:, jb : jb + 1],
                    scale=DESCALE,
                )
            st[(lb, h)] = dict(e8=e8)

        def emit_attn(lb, h):
            s, u = st[lb], st[(lb, h)]
            e8 = u["e8"]
            # rowsum over j via all-ones matmul with M=128: every PSUM
            # partition gets the sum, so no partition broadcast is needed
            # q-outer: rowsum and all four attn accumulators advance as each
            # exp pair lands, so this work interleaves with the exp chain
            # instead of stalling on the last exp (5 concurrent PSUM banks)
            rs = pssp.tile([128, HW], FP32, tag="pss", name=f"psrs{lb}{h}")
            ps_a = [
                pssp.tile([128, HW], FP32, tag="pss", name=f"psa{lb}{h}_{dc}")
                for dc in range(ND)
            ]
            pnv = s["pn8"]
            for q in range(NQ):
                nc.tensor.matmul(
                    rs[:],
                    ones8[:],
                    e8[q][:],
                    start=(q == 0),
                    stop=(q == NQ - 1),
                    perf_mode=DR,
                )
                for dc in range(ND):
                    nc.tensor.matmul(
                        ps_a[dc][:],
                        pnv[:, :, q * D + dc * 128 : q * D + (dc + 1) * 128],
                        e8[q][:],
                        start=(q == 0),
                        stop=(q == NQ - 1),
                        perf_mode=DR,
                    )
            rbb = rbbp.tile([128, HW], FP32, tag="rbb", name=f"rbb{lb}{h}")
            nc.vector.reciprocal_approx_fast(out=rbb[:], in_=rs[:])
            at8 = at8p.tile([128, ND * HW], FP8, tag="at8", name=f"at8_{lb}{h}")
            for dc in range(ND):
                nc.vector.scalar_tensor_tensor(
                    out=at8[:, dc * HW : (dc + 1) * HW],
                    in0=ps_a[dc][:],
                    scalar=2.0,
                    in1=rbb[:],
                    op0=ALU.mult,
                    op1=ALU.mult,
                )
            u["at8"] = at8

        def emit_gates(lb, h):
            s, u = st[lb], st[(lb, h)]
            atv = u["at8"].rearrange("p (c i) -> p c i", i=HW)
            pt16v = s["pt16"][h].rearrange("p (c i) -> p c i", i=HW)
            for ib4 in range(4):
                ib = h * 4 + ib4
                cols = slice(ib4 * 128, (ib4 + 1) * 128)
                ps_g = [
                    psgp.tile([128, 512], FP32, tag="psg",
                              name=f"psg{lb}{h}_{ib4}_{g}")
                    for g in range(3)
                ]
                for g in range(3):
                    # P-half: bf16 for z,r (precision), fp8 DR for f
                    if g < 2:
                        psteps = [
                            ("bf16", pt16v[:, c : c + 1, cols], g * 4 + c)
                            for c in range(ND)
                        ]
                    else:
                        psteps = [
                            ("fp8", s["pt8"][(kp, h)][:, :, cols], 6 + kp)
                            for kp in range(2)
                        ]
                    asteps = [
                        ("fp8", atv[:, 2 * kp : 2 * kp + 2, cols], g * 2 + kp)
                        for kp in range(2)
                    ]
                    steps = psteps + asteps
                    for si, (kind, lhsT, slot) in enumerate(steps):
                        if kind == "bf16":
                            nc.tensor.matmul(
                                ps_g[g],
                                lhsT,
                                wb16[:, slot * D : (slot + 1) * D],
                                start=(si == 0),
                                stop=(si == len(steps) - 1),
                            )
                        else:
                            nc.tensor.matmul(
                                ps_g[g],
                                lhsT,
                                w8slot(slot),
                                start=(si == 0),
                                stop=(si == len(steps) - 1),
                                perf_mode=DR,
                            )
                if with_bias:
                    for g in range(3):
                        nc.vector.tensor_add(ps_g[g][:], ps_g[g][:], bb[g][:])
                z32 = gactp.tile([128, D], FP32, tag="z32")
                r32 = gactp.tile([128, D], FP32, tag="r32")
                f32 = gactp.tile([128, D], FP32, tag="f32")
                nc.scalar.activation(z32[:], ps_g[0][:], AF.Tanh, scale=DESCALE)
                nc.scalar.activation(r32[:], ps_g[1][:], AF.Tanh, scale=DESCALE / 2)
                nc.scalar.activation(f32[:], ps_g[2][:], AF.Tanh, scale=DESCALE / 2)
                # out = (1+r')/2*P + (1+f')/2*z  with r'=tanh(gr/2), f'=tanh(gf/2)
                a32 = combp.tile([128, D], FP32, tag="a32")
                nc.vector.scalar_tensor_tensor(
                    out=a32[:], in0=r32[:], scalar=1.0,
                    in1=s["ph16"][h][:, ib4 * D : (ib4 + 1) * D],
                    op0=ALU.add, op1=ALU.mult,
                )
                b32 = combp.tile([128, D], FP32, tag="b32")
                nc.vector.scalar_tensor_tensor(
                    out=b32[:], in0=f32[:], scalar=1.0, in1=z32[:],
                    op0=ALU.add, op1=ALU.mult,
                )
                o32 = combp.tile([128, D], FP32, tag="o32")
                nc.vector.scalar_tensor_tensor(
                    out=o32[:], in0=b32[:], scalar=0.5, in1=a32[:],
                    op0=ALU.mult, op1=ALU.add,
                )
                # batch 0's outputs ride the otherwise-idle pool ring; the
                # final batch's stay on sync so the last store drains fast
                eng = nc.gpsimd if lb == 0 else nc.sync
                eng.dma_start(out_d[lb, ib * 128 : (ib + 1) * 128, :], o32[:])

        units = [(lb, h) for lb in range(BPC) for h in range(2)]
        for u, (lb, h) in enumerate(units):
            if h == 0:
                emit_load(lb)
            emit_scores(lb, h)
            if u > 0:
                emit_gates(*units[u - 1])
            emit_attn(lb, h)
        emit_gates(*units[-1])

    nc.compile()
    return nc


def _get_nc(with_bias: bool):
    if with_bias not in _cache:
        _cache[with_bias] = _build(with_bias)
    return _cache[with_bias]


def _q8(x):
    return np.clip(np.asarray(x, np.float32), -240.0, 240.0).astype(F8)


def _prep_in_maps(P, w_atten, w1, w2, w3, b1, b2, b3):
    P = np.ascontiguousarray(np.asarray(P, dtype=np.float32))
    w_atten = np.asarray(w_atten, dtype=np.float32)
    wb, wc = w_atten[D : 2 * D], w_atten[2 * D :]

    P8 = _q8(P * 32.0)                       # [B, PL, D] fp8

    # pt8[b, kp, h, p, c2, i] = P8[b, h*512+i, (2kp+c2)*128+p]
    pt8 = np.ascontiguousarray(
        P8.reshape(B, 2, HW, 2, 2, 128).transpose(0, 3, 1, 5, 4, 2)
        .reshape(B, 2, 2, 128, 2 * HW)
    )
    # pwt8[b, kp, p, c2, j] = fp8(P*wc*256)[b, j, (2kp+c2)*128+p]
    PW8 = _q8(P * wc * 256.0)
    pwt8 = np.ascontiguousarray(
        PW8.reshape(B, PL, 2, 2, 128).transpose(0, 2, 4, 3, 1)
        .reshape(B, 2, 128, 2 * PL)
    )
    # pt16[b, h, p, c, i] = bf16(P*32)[b, h*512+i, c*128+p]
    P32b = (P * 32.0).astype(BF)
    pt16 = np.ascontiguousarray(
        P32b.reshape(B, 2, HW, ND, 128).transpose(0, 1, 4, 3, 2)
        .reshape(B, 2, 128, ND * HW)
    )
    pn8 = np.ascontiguousarray(
        P8.reshape(B, NQ, 2, 128, D).transpose(0, 3, 2, 1, 4).reshape(B, 128, 2 * NQ * D)
    )
    # ph16[b, h, p, ib4, e] = bf16(P/2)[b, (h*4+ib4)*128+p, e]
    ph16 = np.ascontiguousarray(
        (P * 0.5).astype(BF).reshape(B, 2, 4, 128, D).transpose(0, 1, 3, 2, 4)
        .reshape(B, 2, 128, 4 * D)
    )
    sjc = np.ascontiguousarray(
        (P @ wb).reshape(B, NI, 128).transpose(0, 2, 1)
    ).astype(np.float32)

    ws = [np.asarray(w, np.float32) for w in (w1, w2, w3)]
    # bf16 P-half weights for gates 0,1: slot g*4+c = w_g[c*128:(c+1)*128]*256
    wb16 = np.zeros((128, NB16, D), dtype=BF)
    for g in range(2):
        wb16[:, g * 4 : (g + 1) * 4] = (
            (ws[g][:D] * 256.0).astype(BF).reshape(ND, 128, D).transpose(1, 0, 2)
        )
    # fp8 slots: 0..5 attn-half pairs (g*2+kp, scale 128); 6,7 g2 P-half (scale 256)
    w8p = np.zeros((128, 2, N8, D), dtype=F8)
    for g in range(3):
        w8p[:, :, g * 2 : (g + 1) * 2] = (
            _q8(ws[g][D:] * 128.0).reshape(2, 2, 128, D).transpose(2, 1, 0, 3)
        )
    w8p[:, :, 6:8] = _q8(ws[2][:D] * 256.0).reshape(2, 2, 128, D).transpose(2, 1, 0, 3)

    biases = np.stack([np.asarray(b, np.float32) for b in (b1, b2, b3)])
    with_bias = bool(np.any(biases))

    base = {
        "wb16": wb16.reshape(128, NB16 * D),
        "w8p": w8p.reshape(128, 2 * N8 * D),
        "ones8": np.ones((128, 2 * 128), dtype=F8),
    }
    if with_bias:
        base["b32"] = biases
    in_maps = []
    for c in range(NCORES):
        sl = slice(c * BPC, (c + 1) * BPC)
        m = dict(base)
        m["pt8"] = pt8[sl]
        m["pwt8"] = pwt8[sl]
        m["pt16"] = pt16[sl]
        m["pn8"] = pn8[sl]
        m["ph16"] = ph16[sl]
        m["sjc"] = sjc[sl]
        in_maps.append(m)
    return in_maps, with_bias


def run(P, w_atten, w1, w2, w3, b1, b2, b3, trace=False):
    in_maps, with_bias = _prep_in_maps(P, w_atten, w1, w2, w3, b1, b2, b3)
    nc = _get_nc(with_bias)
    res = run_bass_kernel_spmd(
        nc, in_maps, core_ids=list(range(NCORES)), trace=trace
    )
    out = np.concatenate([res.results[c]["out"] for c in range(NCORES)], axis=0)
    return out, res


def kernel(P, w_atten, w1, w2, w3, b1, b2, b3):
    out, _ = run(P, w_atten, w1, w2, w3, b1, b2, b3)
    return out
